# revision 5
# baseline (speedup 1.0000x reference)
"""Per-pixel predicted 5x5 conv (KPN-style) on 8 trn2 cores.

Sharding: data-parallel over (batch x H-half) = 8 shards, halo rows included
in each shard's input slice (host-side zero-padded, so no edge cases).

Device layout (per core):
  partitions = 128 output rows (h), free = (c, w) c-major.
  - 10 SBUF copies of the feat slice: 5 h-shifts (di) x 2 w-parities, so every
    tap (di, dj) is a clean slice with 4B-aligned, stride-1 inner w runs ->
    DVE tensor_tensor runs in 2x_1P bf16 mode.
  - per tap: DVE bf16 multiply prod = feat_shift * kernel_tap (kernel tap
    broadcast across c via stride-0 AP dim).
  - 25-tap accumulation: PE identity-matmul PSUM accumulate (start on a bias
    matmul, so bias rides along for free).
  - ACT evacuates PSUM -> SBUF fp32, DMA out.
"""

import sys

for p in ("/opt/pypackages", "/opt/trn_rl_repo"):
    if p not in sys.path:
        sys.path.insert(0, p)

import numpy as np
import ml_dtypes

import concourse.mybir as mybir
from concourse import bacc, tile
from concourse.bass_utils import run_bass_kernel_spmd

B, H, W, C, KK, K = 4, 256, 256, 32, 25, 5
HS = H // 2          # 128 output rows per core
WPAD = W + 8         # w index j == original w (j-2); zeros outside
CH = 16              # channels per half-pass (SBUF fit)
CQ = 8               # channels per PSUM chunk (4 banks)
BF16 = mybir.dt.bfloat16
F32 = mybir.dt.float32

_NC_CACHE = {}


def _build_nc():
    nc = bacc.Bacc(None, target_bir_lowering=False)
    feat_d = nc.dram_tensor("feat", [HS + 4, C, WPAD], BF16, kind="ExternalInput")
    kern_d = nc.dram_tensor("kern", [HS, KK, W], BF16, kind="ExternalInput")
    bias_d = nc.dram_tensor("biasr", [128, C, W], BF16, kind="ExternalInput")
    iden_d = nc.dram_tensor("iden", [128, 128], BF16, kind="ExternalInput")
    out_d = nc.dram_tensor("out", [HS, C, W], F32, kind="ExternalOutput")

    with tile.TileContext(nc) as tc:
        with tc.tile_pool(name="const", bufs=1) as cpool, \
             tc.tile_pool(name="copies", bufs=2) as fpool, \
             tc.tile_pool(name="prod", bufs=6) as ppool, \
             tc.tile_pool(name="osb", bufs=4) as opool, \
             tc.tile_pool(name="psum", bufs=2, space="PSUM") as qpool:
            ident = cpool.tile([128, 128], BF16, tag="ident")
            nc.sync.dma_start(out=ident, in_=iden_d[:, :])
            kern_t = cpool.tile([128, KK, W], BF16, tag="kern")
            nc.sync.dma_start(out=kern_t, in_=kern_d[:, :, :])
            bias_t = cpool.tile([128, C, W], BF16, tag="bias")
            nc.sync.dma_start(out=bias_t, in_=bias_d[:, :, :])

            for qp in range(C // CQ):          # quarter-pass = one PSUM chunk
                cq0 = qp * CQ
                cops = {}
                for di in range(K):
                    for par in range(2):
                        t = fpool.tile([128, CQ, W + 4], BF16,
                                       tag=f"cop{di}_{par}")
                        nc.sync.dma_start(
                            out=t,
                            in_=feat_d[di:di + 128, cq0:cq0 + CQ,
                                       par:par + W + 4])
                        cops[(di, par)] = t
                psum_t = qpool.tile([128, 4, 512], F32, tag="ps")
                # bias seeds the accumulation group (start=True)
                for j in range(4):
                    nc.tensor.matmul(
                        psum_t[:, j:j + 1, :],
                        ident,
                        bias_t[:, cq0 + 2 * j:cq0 + 2 * j + 2, :],
                        start=True, stop=False)
                for ti in range(KK):
                    di, dj = ti // K, ti % K
                    par = dj % 2
                    s = dj - par
                    cop = cops[(di, par)]
                    prod = ppool.tile([128, CQ, W], BF16, tag="prod")
                    in0 = cop[:, :, s:s + W]
                    in1 = kern_t[:, ti:ti + 1, :].broadcast_to(
                        (128, CQ, W))
                    nc.vector.tensor_tensor(prod, in0, in1,
                                            mybir.AluOpType.mult)
                    last = ti == KK - 1
                    for j in range(4):
                        nc.tensor.matmul(
                            psum_t[:, j:j + 1, :],
                            ident,
                            prod[:, 2 * j:2 * j + 2, :],
                            start=False, stop=last)
                for j in range(4):
                    out_sb = opool.tile([128, 2, W], F32, tag="osb")
                    nc.scalar.copy(
                        out=out_sb.rearrange("p a b -> p (a b)"),
                        in_=psum_t[:, j:j + 1, :].rearrange(
                            "p a b -> p (a b)"))
                    nc.sync.dma_start(
                        out=out_d[:, cq0 + 2 * j:cq0 + 2 * j + 2, :],
                        in_=out_sb)
    if not nc.is_finalized():
        nc.finalize()
    return nc


def _get_nc():
    if "nc" not in _NC_CACHE:
        _NC_CACHE["nc"] = _build_nc()
    return _NC_CACHE["nc"]


def _prep_inputs(feat, kernel, bias):
    ft = np.ascontiguousarray(feat.transpose(0, 1, 3, 2))   # [B, H, C, W]
    fp = np.zeros((B, H + 4, C, WPAD), np.float32)
    fp[:, 2:H + 2, :, 2:W + 2] = ft
    fpb = fp.astype(ml_dtypes.bfloat16)
    kt = np.ascontiguousarray(
        kernel.transpose(0, 1, 3, 2)).astype(ml_dtypes.bfloat16)  # [B,H,25,W]
    biasr = np.ascontiguousarray(
        np.broadcast_to(
            bias.astype(ml_dtypes.bfloat16)[None, :, None], (128, C, W)))
    iden = np.eye(128, dtype=ml_dtypes.bfloat16)
    in_maps = []
    for core in range(8):
        b, hh = core // 2, core % 2
        h0 = hh * HS
        in_maps.append({
            "feat": np.ascontiguousarray(fpb[b, h0:h0 + HS + 4]),
            "kern": np.ascontiguousarray(kt[b, h0:h0 + HS]),
            "biasr": biasr,
            "iden": iden,
        })
    return in_maps


def _run(feat, kernel, bias, **run_kwargs):
    nc = _get_nc()
    in_maps = _prep_inputs(feat, kernel, bias)
    res = run_bass_kernel_spmd(nc, in_maps, core_ids=list(range(8)),
                               **run_kwargs)
    out = np.empty((B, H, C, W), np.float32)
    for core in range(8):
        b, hh = core // 2, core % 2
        out[b, hh * HS:(hh + 1) * HS] = res.results[core]["out"]
    return np.ascontiguousarray(out.transpose(0, 1, 3, 2)), res


def kernel(feat, kernel, bias):
    out, _ = _run(np.asarray(feat, np.float32), np.asarray(kernel, np.float32),
                  np.asarray(bias, np.float32))
    return out
